# revision 14
# baseline (speedup 1.0000x reference)
"""Trainium2 Bass kernel for 16-head self-attention (B=4, L=2048, D=1024).

Sharding: 8 cores = 4 batches x 2 head-groups (8 heads each). Each core
computes qkv projection, attention and a partial out-projection for its
(batch, head-group); the host sums the two head-group partials per batch.

Pipeline (per core, single TileContext):
  prologue: k-proj + v-proj (v in per-head [L, 64|ones] aug layout)
  per i-block (512 queries):
    q-proj for the block
    out-proj of the PREVIOUS i-block (lagged, hides under exp)
    per head-pair c: 16 key-blocks of
      score MMs (row-tiled pair, K=64 each, concurrent) ->
      exp split between ACT (table exp) and a custom DVE op
      (cubic q(x/4)^4 approximation, 8 ALU slices, 1 instr/tile) ->
      v-MM pair with ones column accumulating out + softmax denom
    normalize: reciprocal (DVE) -> DRAM-bounce broadcast -> mul (GPSIMD),
    lagged one head-pair to hide the bounce latency.

The softmax exp is the serial bottleneck (256 [128,1024] tiles/core);
splitting it across ScalarE and VectorE is the main win vs the
all-phases-serial baseline.
"""
import sys

sys.path.insert(0, "/opt/trn_rl_repo")

import numpy as np
import ml_dtypes

import concourse.bacc as bacc
import concourse.mybir as mybir
import concourse.tile as tile
from concourse.bass_utils import run_bass_kernel_spmd

F32 = mybir.dt.float32
BF16 = mybir.dt.bfloat16
EXP = mybir.ActivationFunctionType.Exp

B = 4
L = 2048
D = 1024
HEADS = 16
DH = 64
SCALE = DH ** -0.5
N_CORES = 8
NPAIR = 4                # head pairs per core
ND = D // 128            # 8 d-chunks
NL = L // 128            # 16 l-chunks
NIB = 4                  # i-blocks of 512 queries

# --- custom DVE exp: exp(x*SCALE) ~= q(x/32)^4, q cubic (minimax,
# relative-weighted, fitted on score*SCALE in [-3.45, 3.35]) ---
_QC = (0.99938307, 0.99312823, 0.52097751, 0.19024022)  # c0..c3 of q(t)
C0P = _QC[0]                   # imm2
C1P = _QC[1] / 32.0            # s1
C2P = _QC[2] / (32.0 ** 2)     # s0
C3P = _QC[3] / (32.0 ** 3)     # via in1 [P,1]
# key-blocks (of 16) whose exp runs on the DVE instead of ACT
DVE_JS = frozenset((1, 3, 5, 7, 9, 11, 13))

_CACHE = {}


def _exp_dve_op():
    """Create + register the custom DVE exp op (idempotent)."""
    if "op" in _CACHE:
        return _CACHE["op"]
    from concourse.dve_spec import Spec, Src0, C0, C1, C2, C3, sq, lower, \
        _spill_c3_to_src1
    from concourse.dve_uop import DveOpSpec
    from concourse import dve_ops as dops

    body = _spill_c3_to_src1(
        sq(sq(((C3 * Src0 + C0) * Src0 + C1) * Src0 + C2)))

    def _ref(in0, in1, s0, s1, imm2):
        q = ((in1 * in0 + s0) * in0 + s1) * in0 + imm2
        return ((q * q) ** 2).astype(np.float32)

    spec = Spec(body=body, reference=_ref)
    name = "EXP_P3Q_ANT"
    existing = [o for o in dops.OPS if o.name == name]
    if existing:
        _CACHE["op"] = existing[0]
        return existing[0]
    row = dops._CUSTOM_DVE_ROW_BASE + len(dops.OPS)
    shas = {}
    for ver in ("v3", "v4"):
        s = DveOpSpec(name=name, opcode=row, uops=lower(spec, ver=ver),
                      rd1_en=True)
        shas[ver] = s.sha(ver)
    op = dops.DveOp(name, spec, subdim=False, uops_sha=shas)
    dops.OPS.append(op)
    dops._SUB_OPCODE_FOR_NAME[name] = row
    dops.CUSTOM_DVE_SPECS[name] = spec
    _CACHE["op"] = op
    return op


def _build():
    exp_op = _exp_dve_op()
    nc = bacc.Bacc("TRN2", target_bir_lowering=False)

    xT_d = nc.dram_tensor("xT", [D, L], BF16, kind="ExternalInput")
    wqk_d = nc.dram_tensor("wqk", [D, 1024], BF16, kind="ExternalInput")
    wv_d = nc.dram_tensor("wv", [D, 512], BF16, kind="ExternalInput")
    wout_d = nc.dram_tensor("wout", [512, 1024], BF16, kind="ExternalInput")
    bqk_d = nc.dram_tensor("bqk", [8, 128, 1], F32, kind="ExternalInput")
    y_d = nc.dram_tensor("y", [L, D], F32, kind="ExternalOutput")

    with tile.TileContext(nc) as tc:
        with (
            tc.tile_pool(name="persist", bufs=1) as pp,
            tc.tile_pool(name="dstage", bufs=4, space="DRAM") as dpool,
            tc.tile_pool(name="ps", bufs=2, space="PSUM") as ps,
            tc.tile_pool(name="ppool", bufs=3) as ppl,
            tc.tile_pool(name="npool", bufs=3) as npl,
            tc.tile_pool(name="ystage", bufs=3) as ysp,
        ):
            # persistent tiles
            xt = [pp.tile([128, L], BF16, tag=f"xt{d}", name=f"xt{d}")
                  for d in range(ND)]
            wqk = [pp.tile([128, 1024], BF16, tag=f"wqk{d}", name=f"wqk{d}")
                   for d in range(ND)]
            wv = [pp.tile([128, 512], BF16, tag=f"wv{d}", name=f"wv{d}")
                  for d in range(ND)]
            wout = [pp.tile([128, 1024], BF16, tag=f"wo{c}", name=f"wo{c}")
                    for c in range(NPAIR)]
            kt = [pp.tile([128, L], BF16, tag=f"kt{c}", name=f"kt{c}")
                  for c in range(NPAIR)]
            qt = [pp.tile([128, L], BF16, tag=f"qt{c}", name=f"qt{c}")
                  for c in range(NPAIR)]
            vt = [pp.tile([128, 130 * NPAIR], BF16, tag=f"v{l}", name=f"v{l}")
                  for l in range(NL)]
            ot = [pp.tile([128, L], BF16, tag=f"ot{c}", name=f"ot{c}")
                  for c in range(NPAIR)]
            bias = [pp.tile([128, 1], F32, tag=f"b{t}", name=f"b{t}")
                    for t in range(8)]
            ones_f = pp.tile([128, 1], F32, tag="ones")
            c3t = pp.tile([128, 1], F32, tag="c3t")

            nc.vector.memset(ones_f[:], 1.0)
            nc.vector.memset(c3t[:], C3P)
            for t in range(8):
                nc.sync.dma_start(bias[t][:], bqk_d[t])
            for d in range(ND):
                nc.sync.dma_start(xt[d][:], xT_d[d * 128:(d + 1) * 128, :])
                nc.sync.dma_start(wqk[d][:], wqk_d[d * 128:(d + 1) * 128, :])
            for d in range(ND):
                nc.sync.dma_start(wv[d][:], wv_d[d * 128:(d + 1) * 128, :])
            for c in range(NPAIR):
                nc.sync.dma_start(wout[c][:], wout_d[c * 128:(c + 1) * 128, :])

            # ---------------- prologue: k-proj ----------------
            # k chunk c uses wqk cols 512 + c*128, bias index 4+c
            for c in range(NPAIR):
                psn = []
                for n in range(4):
                    t_ = ps.tile([128, 512], F32, tag=("oh", "og")[n % 2],
                                 name=f"kps{c}{n}")
                    psn.append(t_)
                for d in range(ND):
                    w = wqk[d][:, 512 + c * 128:512 + (c + 1) * 128]
                    for n in range(4):
                        nc.tensor.matmul(
                            psn[n][:], w, xt[d][:, n * 512:(n + 1) * 512],
                            start=(d == 0), stop=(d == ND - 1))
                for n in range(4):
                    nc.vector.tensor_scalar_add(
                        kt[c][:, n * 512:(n + 1) * 512], psn[n][:],
                        bias[4 + c][:])

            # ---------------- prologue: v-proj ----------------
            for l in range(NL):
                psv = ps.tile([128, 512], F32, tag=("oh", "og")[l % 2],
                              name=f"vps{l}")
                for d in range(ND):
                    nc.tensor.matmul(
                        psv[:], xt[d][:, l * 128:(l + 1) * 128], wv[d][:],
                        start=(d == 0), stop=(d == ND - 1))
                v3 = vt[l][:].rearrange("p (h w) -> p h w", w=65)
                nc.vector.tensor_copy(
                    v3[:, :, 0:64],
                    psv[:].rearrange("p (h w) -> p h w", w=64))
                nc.vector.tensor_copy(
                    v3[:, :, 64:65],
                    ones_f[:, None, :].broadcast_to([128, 8, 1]))

            # ---------------- main loop over i-blocks ----------------
            pending = []   # lagged normalize muls (hide bounce latency)

            def qproj(ib):
                for c in range(NPAIR):
                    psq = ps.tile([128, 512], F32, tag=("oh", "og")[c % 2],
                                  name=f"qps{ib}{c}")
                    for d in range(ND):
                        nc.tensor.matmul(
                            psq[:], wqk[d][:, c * 128:(c + 1) * 128],
                            xt[d][:, ib * 512:(ib + 1) * 512],
                            start=(d == 0), stop=(d == ND - 1))
                    nc.vector.tensor_scalar_add(
                        qt[c][:, ib * 512:(ib + 1) * 512], psq[:], bias[c][:])

            def attention(c, ib, filler=None):
                isl = slice(ib * 512, (ib + 1) * 512)
                o_h = ps.tile([128, 512], F32, tag="oh", name=f"oh{c}{ib}")
                o_g = ps.tile([128, 512], F32, tag="og", name=f"og{c}{ib}")
                for j in range(NL):
                    if filler is not None and j % 4 == 3:
                        filler(j // 4)
                    js = slice(j * 128, (j + 1) * 128)
                    s = ps.tile([128, 1024], F32, tag="s", name="s")
                    nc.tensor.matmul(s[:, 0:512], kt[c][0:64, js],
                                     qt[c][0:64, isl], start=True, stop=True)
                    nc.tensor.matmul(s[:, 512:1024], kt[c][64:128, js],
                                     qt[c][64:128, isl], start=True, stop=True)
                    p = ppl.tile([128, 1024], BF16, tag="p", name="p")
                    if j in DVE_JS:
                        nc.vector._custom_dve(
                            exp_op, out=p[:], in0=s[:], in1=c3t[:],
                            s0=C2P, s1=C1P, imm2=C0P)
                    else:
                        nc.scalar.activation(p[:], s[:], EXP,
                                             scale=float(SCALE))
                    st, sp = j == 0, j == NL - 1
                    va = vt[j][:, c * 130:c * 130 + 65]
                    vb = vt[j][:, c * 130 + 65:c * 130 + 130]
                    nc.tensor.matmul(o_h[0:65, :], va, p[:, 0:512],
                                     start=st, stop=sp)
                    nc.tensor.matmul(o_g[0:65, :], vb, p[:, 512:1024],
                                     start=st, stop=sp)
                # normalize: reciprocal + bounce DMAs now; the ot muls are
                # lagged one head-pair so the mul never heads the vector
                # FIFO while its broadcast DMA is still in flight.
                rbs = []
                for k, o_t in enumerate((o_h, o_g)):
                    tg = ("rch", "rcg")[k]
                    dcp = npl.tile([1, 512], F32, tag=tg + "d", name=tg + "d")
                    nc.vector.tensor_copy(dcp[:], o_t[64:65, :])
                    rcp = npl.tile([1, 512], F32, tag=tg, name=tg)
                    nc.vector.reciprocal_approx_fast(out=rcp[:], in_=dcp[:])
                    dst = dpool.tile([1, 512], F32, tag=tg)
                    nc.sync.dma_start(dst[:], rcp[:])
                    rb = npl.tile([64, 512], F32, tag=tg + "b", name=tg + "b")
                    nc.sync.dma_start(rb[:], dst[:].to_broadcast([64, 512]))
                    rbs.append(rb)

                def finish(c=c, isl=isl, o_h=o_h, o_g=o_g, rbs=rbs):
                    nc.vector.tensor_mul(ot[c][0:64, isl], o_h[0:64, :],
                                         rbs[0][:])
                    nc.vector.tensor_mul(ot[c][64:128, isl], o_g[0:64, :],
                                         rbs[1][:])
                pending.append(finish)

            def outproj_isub(ib, i):
                # one 128-token row of y: a [128,1024] psum from the s ring,
                # both column-halves accumulating over head pairs
                i0 = ib * 4 + i
                psm = ps.tile([128, 1024], F32, tag="s", name=f"y{i0}")
                for c in range(NPAIR):
                    och = ot[c][:, i0 * 128:(i0 + 1) * 128]
                    nc.tensor.matmul(psm[:, 0:512], och, wout[c][:, 0:512],
                                     start=(c == 0), stop=(c == NPAIR - 1))
                    nc.tensor.matmul(psm[:, 512:1024], och,
                                     wout[c][:, 512:1024],
                                     start=(c == 0), stop=(c == NPAIR - 1))
                yst = ysp.tile([128, 1024], F32, tag="yst", name="yst")
                nc.vector.tensor_copy(yst[:], psm[:])
                nc.sync.dma_start(y_d[i0 * 128:(i0 + 1) * 128, :], yst[:])

            for ib in range(NIB):
                qproj(ib)
                for c in range(NPAIR):
                    # previous i-block's out-proj rides inside the second
                    # head-pair's attention at j-boundaries, so the tensor
                    # queue never starves ACT/DVE of score tiles
                    filler = (lambda i, ib=ib: outproj_isub(ib - 1, i)) \
                        if (c == 1 and ib > 0) else None
                    attention(c, ib, filler)
                    while len(pending) > 1:
                        pending.pop(0)()
            while pending:
                pending.pop(0)()
            for i in range(4):
                outproj_isub(NIB - 1, i)

    nc.finalize()
    return nc


def _get_nc():
    if "nc" not in _CACHE:
        _CACHE["nc"] = _build()
    return _CACHE["nc"]


def _make_in_maps(x, W_qkv, b_qkv, W_out):
    xT = [np.ascontiguousarray(x[b].T).astype(ml_dtypes.bfloat16)
          for b in range(B)]
    in_maps = []
    for b in range(B):
        for g in range(2):
            sl = slice(g * 512, (g + 1) * 512)
            wqk_c = np.ascontiguousarray(
                np.concatenate([W_qkv[:, sl],
                                W_qkv[:, 1024 + g * 512:1024 + (g + 1) * 512]],
                               axis=1)).astype(ml_dtypes.bfloat16)
            wv_c = np.ascontiguousarray(
                W_qkv[:, 2048 + g * 512:2048 + (g + 1) * 512]).astype(
                    ml_dtypes.bfloat16)
            wout_c = np.ascontiguousarray(W_out[sl, :]).astype(
                ml_dtypes.bfloat16)
            bqk_c = np.concatenate(
                [b_qkv[g * 512:(g + 1) * 512],
                 b_qkv[1024 + g * 512:1024 + (g + 1) * 512]]).reshape(8, 128, 1)
            in_maps.append({
                "xT": xT[b],
                "wqk": wqk_c,
                "wv": wv_c,
                "wout": wout_c,
                "bqk": np.ascontiguousarray(bqk_c),
            })
    return in_maps


def kernel(x, W_qkv, b_qkv, W_out, b_out):
    x = np.asarray(x, dtype=np.float32)
    W_qkv = np.asarray(W_qkv, dtype=np.float32)
    b_qkv = np.asarray(b_qkv, dtype=np.float32)
    W_out = np.asarray(W_out, dtype=np.float32)
    b_out = np.asarray(b_out, dtype=np.float32)

    nc = _get_nc()
    in_maps = _make_in_maps(x, W_qkv, b_qkv, W_out)
    res = run_bass_kernel_spmd(nc, in_maps, core_ids=list(range(N_CORES)))

    # v-bias flows additively through softmax (rows sum to 1): + b_v @ W_out
    y_bias = b_qkv[2048:3072] @ W_out + b_out
    out = np.empty((B, L, D), dtype=np.float32)
    for b in range(B):
        out[b] = res.results[2 * b]["y"] + res.results[2 * b + 1]["y"] + y_bias
    return out


# revision 24
# speedup vs baseline: 1.5951x; 1.5951x over previous
"""Trainium2 Bass kernel for 16-head self-attention (B=4, L=2048, D=1024).

Sharding: 8 cores = 4 batches x 2 head-groups (8 heads each). Each core
computes qkv projection, attention and a partial out-projection for its
(batch, head-group); the host sums the two head-group partials per batch.

Pipeline (per core, single TileContext):
  prologue: k-proj + v-proj (v in per-head [L, 64|ones] aug layout)
  per i-block (512 queries):
    q-proj for the block
    out-proj of the PREVIOUS i-block (lagged, hides under exp)
    per head-pair c: 16 key-blocks of
      score MMs (row-tiled pair, K=64 each, concurrent) ->
      exp split between ACT (table exp) and a custom DVE op
      (cubic q(x/4)^4 approximation, 8 ALU slices, 1 instr/tile) ->
      v-MM pair with ones column accumulating out + softmax denom
    normalize: reciprocal (DVE) -> DRAM-bounce broadcast -> mul (GPSIMD),
    lagged one head-pair to hide the bounce latency.

The softmax exp is the serial bottleneck (256 [128,1024] tiles/core);
splitting it across ScalarE and VectorE is the main win vs the
all-phases-serial baseline.
"""
import sys

sys.path.insert(0, "/opt/trn_rl_repo")

import numpy as np
import ml_dtypes

import concourse.bacc as bacc
import concourse.mybir as mybir
import concourse.tile as tile
from concourse.bass_utils import run_bass_kernel_spmd

F32 = mybir.dt.float32
BF16 = mybir.dt.bfloat16
EXP = mybir.ActivationFunctionType.Exp

B = 4
L = 2048
D = 1024
HEADS = 16
DH = 64
SCALE = DH ** -0.5
N_CORES = 8
NPAIR = 4                # head pairs per core
ND = D // 128            # 8 d-chunks
NL = L // 128            # 16 l-chunks
NIB = 4                  # i-blocks of 512 queries

# --- custom DVE exp: exp(x*SCALE) ~= q(x/32)^4, q cubic (minimax,
# relative-weighted, fitted on score*SCALE in [-3.45, 3.35]) ---
_QC = (0.99938307, 0.99312823, 0.52097751, 0.19024022)  # c0..c3 of q(t)
C0P = _QC[0]                   # imm2
C1P = _QC[1] / 32.0            # s1
C2P = _QC[2] / (32.0 ** 2)     # s0
C3P = _QC[3] / (32.0 ** 3)     # via in1 [P,1]
# key-blocks (of 16) whose exp runs on the DVE instead of ACT
DVE_JS = frozenset((1, 4, 6, 9, 11, 14))

_CACHE = {}


def _exp_dve_op():
    """Create + register the custom DVE exp op (idempotent)."""
    if "op" in _CACHE:
        return _CACHE["op"]
    from concourse.dve_spec import Spec, Src0, C0, C1, C2, C3, sq, lower, \
        _spill_c3_to_src1
    from concourse.dve_uop import DveOpSpec
    from concourse import dve_ops as dops

    body = _spill_c3_to_src1(
        sq(sq(((C3 * Src0 + C0) * Src0 + C1) * Src0 + C2)))

    def _ref(in0, in1, s0, s1, imm2):
        q = ((in1 * in0 + s0) * in0 + s1) * in0 + imm2
        return ((q * q) ** 2).astype(np.float32)

    spec = Spec(body=body, reference=_ref)
    name = "EXP_P3Q_ANT"
    existing = [o for o in dops.OPS if o.name == name]
    if existing:
        _CACHE["op"] = existing[0]
        return existing[0]
    row = dops._CUSTOM_DVE_ROW_BASE + len(dops.OPS)
    shas = {}
    for ver in ("v3", "v4"):
        s = DveOpSpec(name=name, opcode=row, uops=lower(spec, ver=ver),
                      rd1_en=True)
        shas[ver] = s.sha(ver)
    op = dops.DveOp(name, spec, subdim=False, uops_sha=shas)
    dops.OPS.append(op)
    dops._SUB_OPCODE_FOR_NAME[name] = row
    dops.CUSTOM_DVE_SPECS[name] = spec
    _CACHE["op"] = op
    return op


def _build():
    exp_op = _exp_dve_op()
    nc = bacc.Bacc("TRN2", target_bir_lowering=False)

    xT_d = nc.dram_tensor("xT", [D, L], BF16, kind="ExternalInput")
    wqk_d = nc.dram_tensor("wqk", [D, 1024], BF16, kind="ExternalInput")
    wv_d = nc.dram_tensor("wv", [D, 512], BF16, kind="ExternalInput")
    wout_d = nc.dram_tensor("wout", [512, 1024], BF16, kind="ExternalInput")
    bqk_d = nc.dram_tensor("bqk", [8, 128, 1], F32, kind="ExternalInput")
    y_d = nc.dram_tensor("y", [L, D], F32, kind="ExternalOutput")

    with tile.TileContext(nc) as tc:
        with (
            tc.tile_pool(name="persist", bufs=1) as pp,
            tc.tile_pool(name="dstage", bufs=4, space="DRAM") as dpool,
            tc.tile_pool(name="ps", bufs=2, space="PSUM") as ps,
            tc.tile_pool(name="ppool", bufs=3) as ppl,
            tc.tile_pool(name="npool", bufs=3) as npl,
            tc.tile_pool(name="ystage", bufs=3) as ysp,
        ):
            # persistent tiles
            xt = [pp.tile([128, L], BF16, tag=f"xt{d}", name=f"xt{d}")
                  for d in range(ND)]
            wqk = [pp.tile([128, 1024], BF16, tag=f"wqk{d}", name=f"wqk{d}")
                   for d in range(ND)]
            wv = [pp.tile([128, 512], BF16, tag=f"wv{d}", name=f"wv{d}")
                  for d in range(ND)]
            wout = [pp.tile([128, 1024], BF16, tag=f"wo{c}", name=f"wo{c}")
                    for c in range(NPAIR)]
            kt = [pp.tile([128, L], BF16, tag=f"kt{c}", name=f"kt{c}")
                  for c in range(NPAIR)]
            qt = [pp.tile([128, L], BF16, tag=f"qt{c}", name=f"qt{c}")
                  for c in range(NPAIR)]
            vt = [pp.tile([128, 130 * NPAIR], BF16, tag=f"v{l}", name=f"v{l}")
                  for l in range(NL)]
            ot = [pp.tile([128, L], BF16, tag=f"ot{c}", name=f"ot{c}")
                  for c in range(NPAIR)]
            bias = [pp.tile([128, 1], F32, tag=f"b{t}", name=f"b{t}")
                    for t in range(8)]
            ones_f = pp.tile([128, 1], F32, tag="ones")
            c3t = pp.tile([128, 1], F32, tag="c3t")

            nc.vector.memset(ones_f[:], 1.0)
            nc.vector.memset(c3t[:], C3P)
            for t in range(8):
                nc.sync.dma_start(bias[t][:], bqk_d[t])
            for d in range(ND):
                nc.sync.dma_start(xt[d][:], xT_d[d * 128:(d + 1) * 128, :])
                nc.sync.dma_start(wqk[d][:], wqk_d[d * 128:(d + 1) * 128, :])
            for d in range(ND):
                nc.sync.dma_start(wv[d][:], wv_d[d * 128:(d + 1) * 128, :])
            for c in range(NPAIR):
                nc.sync.dma_start(wout[c][:], wout_d[c * 128:(c + 1) * 128, :])

            # ---------------- prologue: k-proj ----------------
            # k chunk c uses wqk cols 512 + c*128, bias index 4+c
            for c in range(NPAIR):
                psA = ps.tile([128, 1024], F32, tag="s", name=f"kpa{c}")
                psB = ps.tile([128, 1024], F32, tag="o", name=f"kpb{c}")
                halves = [psA[:, 0:512], psA[:, 512:1024],
                          psB[:, 0:512], psB[:, 512:1024]]
                for d in range(ND):
                    w = wqk[d][:, 512 + c * 128:512 + (c + 1) * 128]
                    for n in range(4):
                        nc.tensor.matmul(
                            halves[n], w, xt[d][:, n * 512:(n + 1) * 512],
                            start=(d == 0), stop=(d == ND - 1))
                for n in range(4):
                    nc.vector.tensor_scalar_add(
                        kt[c][:, n * 512:(n + 1) * 512], halves[n],
                        bias[4 + c][:])

            # ---------------- prologue: v-proj ----------------
            for l in range(NL):
                psv = ps.tile([128, 512], F32, tag=("s", "o")[l % 2],
                              name=f"vps{l}")
                for d in range(ND):
                    nc.tensor.matmul(
                        psv[:], xt[d][:, l * 128:(l + 1) * 128], wv[d][:],
                        start=(d == 0), stop=(d == ND - 1))
                v3 = vt[l][:].rearrange("p (h w) -> p h w", w=65)
                nc.vector.tensor_copy(
                    v3[:, :, 0:64],
                    psv[:].rearrange("p (h w) -> p h w", w=64))
                nc.vector.tensor_copy(
                    v3[:, :, 64:65],
                    ones_f[:, None, :].broadcast_to([128, 8, 1]))

            # ---------------- main loop over i-blocks ----------------
            pending = []   # lagged normalize muls (hide bounce latency)

            def qproj(ib):
                for c in range(NPAIR):
                    psq = ps.tile([128, 512], F32, tag=("s", "o")[c % 2],
                                  name=f"qps{ib}{c}")
                    for d in range(ND):
                        nc.tensor.matmul(
                            psq[:], wqk[d][:, c * 128:(c + 1) * 128],
                            xt[d][:, ib * 512:(ib + 1) * 512],
                            start=(d == 0), stop=(d == ND - 1))
                    nc.vector.tensor_scalar_add(
                        qt[c][:, ib * 512:(ib + 1) * 512], psq[:], bias[c][:])

            def attention(c, ib):
                isl = slice(ib * 512, (ib + 1) * 512)
                # one [128,1024] psum: cols 0:512 head h, 512:1024 head h';
                # rows 0:64 = outT, row 64 = softmax denominator
                o = ps.tile([128, 1024], F32, tag="o", name=f"o{c}{ib}")
                avq = []   # attnv emitters, lagged 2 iters so the tensor
                # queue never head-blocks on an in-flight exp
                for j in range(NL):
                    js = slice(j * 128, (j + 1) * 128)
                    s = ps.tile([128, 1024], F32, tag="s", name="s")
                    nc.tensor.matmul(s[:, 0:512], kt[c][0:64, js],
                                     qt[c][0:64, isl], start=True, stop=True)
                    nc.tensor.matmul(s[:, 512:1024], kt[c][64:128, js],
                                     qt[c][64:128, isl], start=True, stop=True)
                    p = ppl.tile([128, 1024], BF16, tag="p", name="p")
                    if j in DVE_JS:
                        nc.vector._custom_dve(
                            exp_op, out=p[:], in0=s[:], in1=c3t[:],
                            s0=C2P, s1=C1P, imm2=C0P)
                    else:
                        nc.scalar.activation(p[:], s[:], EXP,
                                             scale=float(SCALE))

                    def attnv(j=j, p=p):
                        st, sp = j == 0, j == NL - 1
                        va = vt[j][:, c * 130:c * 130 + 65]
                        vb = vt[j][:, c * 130 + 65:c * 130 + 130]
                        nc.tensor.matmul(o[0:65, 0:512], va, p[:, 0:512],
                                         start=st, stop=sp)
                        nc.tensor.matmul(o[0:65, 512:1024], vb,
                                         p[:, 512:1024], start=st, stop=sp)
                    avq.append(attnv)
                    if len(avq) > 2:
                        avq.pop(0)()
                while avq:
                    avq.pop(0)()
                # normalize: reciprocal + bounce DMAs now; the ot muls are
                # lagged one head-pair so the mul never heads the vector
                # FIFO while its broadcast DMA is still in flight.
                dcp = npl.tile([1, 1024], F32, tag="dcp", name="dcp")
                nc.vector.tensor_copy(dcp[:], o[64:65, :])
                rcp = npl.tile([1, 1024], F32, tag="rcp", name="rcp")
                nc.vector.reciprocal_approx_fast(out=rcp[:], in_=dcp[:])
                dst = dpool.tile([1, 1024], F32, tag="bnc")
                nc.sync.dma_start(dst[:], rcp[:])
                rb = npl.tile([64, 1024], F32, tag="rb", name="rb")
                nc.sync.dma_start(rb[:], dst[:].to_broadcast([64, 1024]))

                def finish(c=c, isl=isl, o=o, rb=rb):
                    nc.vector.tensor_mul(ot[c][0:64, isl], o[0:64, 0:512],
                                         rb[:, 0:512])
                    nc.vector.tensor_mul(ot[c][64:128, isl],
                                         o[0:64, 512:1024], rb[:, 512:1024])
                pending.append(finish)

            def outproj_isub(ib, i):
                # one 128-token row of y: a [128,1024] psum from the s ring,
                # both column-halves accumulating over head pairs
                i0 = ib * 4 + i
                psm = ps.tile([128, 1024], F32, tag="s", name=f"y{i0}")
                for c in range(NPAIR):
                    och = ot[c][:, i0 * 128:(i0 + 1) * 128]
                    nc.tensor.matmul(psm[:, 0:512], och, wout[c][:, 0:512],
                                     start=(c == 0), stop=(c == NPAIR - 1))
                    nc.tensor.matmul(psm[:, 512:1024], och,
                                     wout[c][:, 512:1024],
                                     start=(c == 0), stop=(c == NPAIR - 1))
                yst = ysp.tile([128, 1024], F32, tag="yst", name="yst")
                nc.vector.tensor_copy(yst[:, 0:512], psm[:, 0:512])
                nc.scalar.copy(yst[:, 512:1024], psm[:, 512:1024])
                nc.sync.dma_start(y_d[i0 * 128:(i0 + 1) * 128, :], yst[:])

            for ib in range(NIB):
                while pending:
                    pending.pop(0)()
                qproj(ib)
                if ib > 0:
                    for i in range(4):
                        outproj_isub(ib - 1, i)
                for c in range(NPAIR):
                    attention(c, ib)
                    while len(pending) > 1:
                        pending.pop(0)()
            while pending:
                pending.pop(0)()
            for i in range(4):
                outproj_isub(NIB - 1, i)

    nc.finalize()
    return nc


def _get_nc():
    if "nc" not in _CACHE:
        _CACHE["nc"] = _build()
    return _CACHE["nc"]


def _make_in_maps(x, W_qkv, b_qkv, W_out):
    xT = [np.ascontiguousarray(x[b].T).astype(ml_dtypes.bfloat16)
          for b in range(B)]
    in_maps = []
    for b in range(B):
        for g in range(2):
            sl = slice(g * 512, (g + 1) * 512)
            wqk_c = np.ascontiguousarray(
                np.concatenate([W_qkv[:, sl],
                                W_qkv[:, 1024 + g * 512:1024 + (g + 1) * 512]],
                               axis=1)).astype(ml_dtypes.bfloat16)
            wv_c = np.ascontiguousarray(
                W_qkv[:, 2048 + g * 512:2048 + (g + 1) * 512]).astype(
                    ml_dtypes.bfloat16)
            wout_c = np.ascontiguousarray(W_out[sl, :]).astype(
                ml_dtypes.bfloat16)
            bqk_c = np.concatenate(
                [b_qkv[g * 512:(g + 1) * 512],
                 b_qkv[1024 + g * 512:1024 + (g + 1) * 512]]).reshape(8, 128, 1)
            in_maps.append({
                "xT": xT[b],
                "wqk": wqk_c,
                "wv": wv_c,
                "wout": wout_c,
                "bqk": np.ascontiguousarray(bqk_c),
            })
    return in_maps


def kernel(x, W_qkv, b_qkv, W_out, b_out):
    x = np.asarray(x, dtype=np.float32)
    W_qkv = np.asarray(W_qkv, dtype=np.float32)
    b_qkv = np.asarray(b_qkv, dtype=np.float32)
    W_out = np.asarray(W_out, dtype=np.float32)
    b_out = np.asarray(b_out, dtype=np.float32)

    nc = _get_nc()
    in_maps = _make_in_maps(x, W_qkv, b_qkv, W_out)
    res = run_bass_kernel_spmd(nc, in_maps, core_ids=list(range(N_CORES)))

    # v-bias flows additively through softmax (rows sum to 1): + b_v @ W_out
    y_bias = b_qkv[2048:3072] @ W_out + b_out
    out = np.empty((B, L, D), dtype=np.float32)
    for b in range(B):
        out[b] = res.results[2 * b]["y"] + res.results[2 * b + 1]["y"] + y_bias
    return out
